# revision 30
# baseline (speedup 1.0000x reference)
"""Trainium2 Bass kernel for nn_LinearKAN (histogram_binning), v2.

Math
----
reference computes, per (batch b, out o):

    out[b,o] = sum_i  PL_interp(x[b,i]; bp[o,i,:], val[o,i,:])

bp is the SAME sorted uniform grid for every (o,i) (tiled linspace).
With u = (x - bp0)/h in [0, S) and uc = u - S/2 in [-10, 10), any
continuous piecewise-linear function on the uniform grid has an exact
*two-sided kink basis* expansion (one ReLU kink per interior knot,
negative-side kinks folded into the affine part):

    f(uc) = a + b*uc + sum_{s'=1..9}  c_{s'} * (max(uc,s')-s')
                     + sum_{s'=-9..-1} d_{s'} * (min(uc,s')-s')

where c/d are second differences of the values (slope changes).  Each
basis tile is ONE tensor_scalar op (max/min then subtract) -- no clamp,
so half the elementwise work of the clamp basis -- and the layer is a
bias plus 20 dense K=128 matmul tiles contracting over (s', i):

    out[b,o] = bias[o] + sum_{s'} sum_i C_{s'}[o,i] * g_{s'}[b,i]

All operands fp16 (basis magnitudes <= 10 by the two-sided centering);
measured ~7e-3 rel err vs the 2e-2 gate.

Device kernel (per core, SPMD over 8 cores, 4 b-quarters x 2 o-halves):
  - ScalarE HWDGE queue DMAs xb (x fp16 + bias col); SyncE queue DMAs
    the C coefficients in 2 chunks; descriptor gen runs in parallel.
  - DVE: uc = scale*x (one 4x fp16 tensor_scalar), then most g tiles
    (194ns each at 4x); ScalarE Relu produces a few g tiles in between.
  - PE: HAM warmup dummies, then 40 fp16 matmuls (K=128 chunks of the
    (s',i) contraction) accumulating one PSUM group.
  - tail: ACT Identity adds bias + casts to fp16, DMA out.
Host only slices/transposes/differences the params (layout prep) and
casts dtypes.
"""

import os
import numpy as np

import concourse.bass as bass
import concourse.mybir as mybir
import concourse.tile as tile
from concourse import bacc
from concourse.bass_utils import run_bass_kernel_spmd

# Problem shape (hardcoded per the task contract).
B, O, I, S = 1024, 256, 256, 20
N_CORES = 8
B_SPLIT, O_SPLIT = 4, 2
B_LOC, O_LOC = B // B_SPLIT, O // O_SPLIT  # 256, 128
SC = S // 2         # grid center; uc = u - SC in [-10, 10)
NT = S              # basis tiles: uc (linear) + 19 kinks
KT = 2 * S          # 40 k-tiles of 128 over the (s', i) contraction
F32 = mybir.dt.float32
F16 = mybir.dt.float16
FW = 2 * B_LOC      # g/uc tile free width: both i-halves side by side
XB_COLS = FW + 2    # xb layout: [x data | bias col | pad]

N_WARMUP_MM = int(os.environ.get("KAN_WARMUP", "0"))  # in-context warmup MMs
N_ACT = int(os.environ.get("KAN_NACT", "5"))          # g tiles built on ACT
# C DMA chunk split in kt units (2 kt per basis tile), smallest first so
# the matmul stream can start as soon as uc is ready.  First chunks ride
# the ScalarE HWDGE ring (behind xb), the rest the SyncE ring.
CHUNK_KT = tuple(int(v) for v in
                 os.environ.get("KAN_CHUNKS", "4,10,12,10,4").split(","))
N_PRE_WARM = int(os.environ.get("KAN_PREWARM", "7"))   # block[0] warmup MMs
# bridge warmup MMs inserted before the first kt of chunk 1, 2, ...
BRIDGE_MM = tuple(int(v) for v in
                  os.environ.get("KAN_BRIDGE", "3,1,1,1").split(","))
# All C chunks ride the SyncE ring: the two HWDGE rings do not share the
# 16 SDMA engines fairly (measured: the busier ring starves the other),
# so the critical small chunks must be FIFO-first on the same ring as the
# big ones.  xb alone rides the ScalarE ring (done before C saturates).
N_SCALAR_CHUNKS = int(os.environ.get("KAN_SCHUNKS", "0"))
assert sum(CHUNK_KT) == KT

# Production/consumption order of basis tiles: uc first, then kinks by
# |s'|.  ACT-assigned tiles sit where their (later) completion lands.
_KINKS = [0]
for m in range(1, SC):
    _KINKS.append(m)
    _KINKS.append(-m)
# positions (0-based among the 19 kinks) handled by ACT
_ACT_POS = {5, 8, 11, 14, 17} if N_ACT == 5 else \
    set(np.linspace(4, 18, max(N_ACT, 1), dtype=int).tolist() if N_ACT else [])
ORDER = [(sp, (j in _ACT_POS)) for j, sp in enumerate(_KINKS)]  # (s', on_act)


def _strip_init_boilerplate(nc) -> None:
    """Drop the Bass-init const-AP memsets + all-engine barrier (~1.5us of
    preamble).  This kernel never reads the const APs (all activation biases
    are explicit APs), so the memsets and their barrier are dead weight."""
    blk = nc.m.functions[0].blocks[0]
    drop = (mybir.InstMemset, mybir.InstDrain, mybir.InstEventSemaphore)
    keep = [i for i in blk.instructions if not isinstance(i, drop)]
    del blk.instructions[:]
    for i in keep:
        blk.instructions.append(i)
    nc.const_aps.aps.clear()


def _build_nc(scale: float, ucbias: float) -> bass.Bass:
    """Build the (SPMD-identical) single-core Bass graph."""
    nc = bacc.Bacc("TRN2", target_bir_lowering=False, debug=False)
    _strip_init_boilerplate(nc)
    if os.environ.get("KAN_NOGPS", "0") == "1":
        # GpSimd runs no instructions in this kernel; dropping it from the
        # engine set removes its instruction-stream load and its rounds in
        # the tile entry/exit barriers.
        nc.engines.pop(mybir.EngineType.Pool, None)

    xb = nc.declare_dram_parameter("xb", [128, XB_COLS], F16, isOutput=False)
    C = nc.declare_dram_parameter("C", [128, KT * 128], F16, isOutput=False)
    out = nc.declare_dram_parameter("out", [O_LOC, B_LOC], F16, isOutput=True)

    # --- pre-context PE HAM warmup: matmuls over (uninitialized) SBUF
    # scratch into a dead PSUM bank.  These sit in the function's first
    # basic block, so the PE array chews them during the runtime boot /
    # instruction-load window (~0.4-5us) and the clock gate opens
    # (1.2 -> 2.4 GHz) long before the real stream.  Garbage operands are
    # harmless: the PSUM bank is never read.
    warm_src = nc.alloc_sbuf_tensor("warm_src", [128, 640], F16, side="right")
    warm_ps = nc.place_psum_tensor("warm_ps", [128, 512], F32, bank=7)
    for _ in range(N_PRE_WARM):
        nc.tensor.matmul(warm_ps[:], warm_src[:, 0:128], warm_src[:, 128:640],
                         start=True, stop=True)

    with tile.TileContext(nc) as tc:
        with (
            tc.tile_pool(name="xb", bufs=1) as xpool,
            tc.tile_pool(name="u", bufs=1) as upool,
            tc.tile_pool(name="w", bufs=3) as wpool,
            tc.tile_pool(name="g", bufs=NT) as gpool,
            tc.tile_pool(name="c", bufs=len(CHUNK_KT)) as cpool,
            tc.tile_pool(name="b", bufs=1) as bpool,
            tc.tile_pool(name="o", bufs=1) as opool,
            tc.tile_pool(name="ps", bufs=2, space="PSUM") as pspool,
        ):
            # --- input DMAs, parallel descriptor-gen on two queues:
            # xb on the ScalarE HWDGE queue (gates the whole DVE chain),
            # C chunks on the SyncE queue.
            # Ring-arming dummies: a 1-descriptor transfer first on each
            # HWDGE ring pays the cold doorbell->first-byte latency
            # (~1.3us) on a throwaway, so the real transfers behind it see
            # the warm-ring latency (~0.6-0.9us).
            if os.environ.get("KAN_ARM", "0") == "1":
                arm = wpool.tile([1, 64], F16, tag="arm")
                nc.scalar.dma_start(arm[:], xb[0:1, 0:64])
                arm2 = wpool.tile([1, 64], F16, tag="arm2")
                nc.sync.dma_start(arm2[:], C[0:1, 0:64])
            xb_sb = xpool.tile([128, XB_COLS], F16)
            nc.scalar.dma_start(xb_sb[:], xb[:])
            ckt = {}
            kt0 = 0
            for ci, nkt in enumerate(CHUNK_KT):
                t = cpool.tile([128, nkt * 128], F16, tag=f"c{ci}")
                eng = nc.scalar if ci < N_SCALAR_CHUNKS else nc.sync
                eng.dma_start(t[:], C[:, kt0 * 128:(kt0 + nkt) * 128])
                for k in range(nkt):
                    ckt[kt0 + k] = t[:, k * 128:(k + 1) * 128]
                kt0 += nkt

            # --- DVE preamble: ACT bias-constant table memsets ---
            ctab = bpool.tile([128, NT], F32, tag="ctab")
            act_bias_col = {}
            for j, (sp, on_act) in enumerate(ORDER):
                if on_act:
                    # relu(uc - s') for s'>0 : bias -s'; relu(s'-uc) for
                    # s'<0 (scale=-1): bias +s'
                    val = -float(sp) if sp >= 0 else float(sp)
                    nc.vector.memset(ctab[:, j:j + 1], val)
                    act_bias_col[j] = ctab[:, j:j + 1]

            # --- in-context bridge warmups: first PE instructions, no
            # upstream deps (raw garbage operands), keep the PE busy from
            # context entry until the real stream starts so the clock gate
            # stays open.
            ps_warm = pspool.tile([128, 512], F32, tag="pw")
            for _ in range(N_WARMUP_MM):
                nc.tensor.matmul(ps_warm[:], warm_src[:, 0:128],
                                 warm_src[:, 128:640],
                                 start=True, stop=True)

            # --- uc = scale*x + ucbias on DVE (4x fp16), both i-halves ---
            uc = upool.tile([128, FW], F16)
            nc.vector.tensor_scalar(
                uc[:], xb_sb[:, 0:FW], float(scale), float(ucbias),
                mybir.AluOpType.mult, mybir.AluOpType.add)

            # --- g tiles: one tensor_scalar / activation each ---
            g = [uc]
            for j, (sp, on_act) in enumerate(ORDER):
                gt = gpool.tile([128, FW], F16, tag="g")
                if on_act:
                    if sp >= 0:
                        nc.scalar.activation(
                            gt[:], uc[:], mybir.ActivationFunctionType.Relu,
                            bias=act_bias_col[j], scale=1.0)
                    else:
                        nc.scalar.activation(
                            gt[:], uc[:], mybir.ActivationFunctionType.Relu,
                            bias=act_bias_col[j], scale=-1.0)
                else:
                    op0 = (mybir.AluOpType.max if sp >= 0
                           else mybir.AluOpType.min)
                    nc.vector.tensor_scalar(
                        gt[:], uc[:], float(sp), float(sp),
                        op0, mybir.AluOpType.subtract)
                g.append(gt)

            # --- matmuls: one PSUM accumulation group, kt = 2*tile + ih.
            # At chunk boundaries, insert dependency-free bridge warmups so
            # the PE stays busy (HAM clock-gate open) while the next C
            # chunk's completion semaphore is still in flight.
            bridge = {}
            kt0 = CHUNK_KT[0]
            for nb, nkt in zip(BRIDGE_MM, CHUNK_KT[1:]):
                bridge[kt0] = nb
                kt0 += nkt
            ps = pspool.tile([O_LOC, B_LOC], F32, tag="ps")
            for kt in range(KT):
                for _ in range(bridge.get(kt, 0)):
                    nc.tensor.matmul(ps_warm[:], warm_src[:, 0:128],
                                     warm_src[:, 128:640],
                                     start=True, stop=True,
                                     skip_group_check=True)
                t, ih = kt // 2, kt % 2
                rhs = g[t][:, ih * B_LOC:(ih + 1) * B_LOC]
                nc.tensor.matmul(ps[:], ckt[kt], rhs,
                                 start=(kt == 0), stop=(kt == KT - 1),
                                 skip_group_check=True)

            # --- tail: out16 = ps + bias, split by partition halves so the
            # ScalarE and VectorE PSUM ports work in parallel and the two
            # HWDGE rings overlap their descriptor-gens.
            out_sb = opool.tile([O_LOC, B_LOC], F16, tag="osb")
            if os.environ.get("KAN_TAIL2", "1") == "1":
                HO = O_LOC // 2
                bias32 = bpool.tile([128, 1], F32, tag="b32")
                nc.vector.tensor_scalar(
                    bias32[:], xb_sb[:, FW:FW + 1], 0.0, None,
                    mybir.AluOpType.add)
                nc.vector.tensor_scalar(
                    out_sb[HO:, :], ps[HO:, :], bias32[HO:, 0:1],
                    None, mybir.AluOpType.add)
                nc.scalar.activation(
                    out_sb[0:HO, :], ps[0:HO, :],
                    mybir.ActivationFunctionType.Identity,
                    bias=xb_sb[0:HO, FW:FW + 1], scale=1.0)
                nc.scalar.dma_start(out[0:HO, :], out_sb[0:HO, :])
                nc.sync.dma_start(out[HO:, :], out_sb[HO:, :])
            else:
                nc.scalar.activation(
                    out_sb[:], ps[:], mybir.ActivationFunctionType.Identity,
                    bias=xb_sb[:, FW:FW + 1], scale=1.0)
                nc.scalar.dma_start(out[:], out_sb[:])
    nc.compile()
    return nc


_NC_CACHE: dict = {}


def _get_nc(scale: float, ucbias: float) -> bass.Bass:
    key = (float(scale), float(ucbias))
    if key not in _NC_CACHE:
        _NC_CACHE[key] = _build_nc(scale, ucbias)
    return _NC_CACHE[key]


def prepare(x: np.ndarray, breakpoints: np.ndarray, values: np.ndarray):
    """Host prep: build the Bass graph (cached) + per-core input maps."""
    x = np.asarray(x, np.float32)
    breakpoints = np.asarray(breakpoints, np.float32)
    values = np.asarray(values, np.float32)

    # Grid affine params from the (shared) breakpoint row.
    bpr = breakpoints[0, 0].astype(np.float64)
    h = (bpr[-1] - bpr[0]) / S
    scale = float(1.0 / h)
    ucbias = float(-bpr[0] / h - SC)

    # Two-sided kink coefficients from the values.
    V = values.astype(np.float64)                    # [O, I, S+1]
    M = V[:, :, 1:] - V[:, :, :-1]                   # [O, I, S] slopes
    c = np.zeros((O, I, S))
    c[:, :, 1:] = M[:, :, 1:] - M[:, :, :-1]         # kinks at knots 1..19
    b_lin = M[:, :, 0] + c[:, :, 1:SC].sum(axis=2)   # affine part after
    a_tot = (V[:, :, 0] - (c[:, :, 1:SC] * np.arange(1, SC)).sum(axis=2)
             + b_lin * SC)                           # folding s'<0 kinks
    bias_o = a_tot.sum(axis=1)                       # [O]

    # Per-tile coefficient planes in production ORDER, sign per engine:
    #  tile j=0: uc            -> b_lin
    #  s'>0 (either engine)    -> +c_s    (basis relu(uc-s'))
    #  s'<0 on DVE             -> -c_s    (basis min(uc,s')-s')
    #  s'<0 on ACT             -> +c_s    (basis relu(s'-uc))
    planes = [b_lin]
    for sp, on_act in ORDER:
        s = sp + SC
        planes.append(c[:, :, s] if (sp >= 0 or on_act) else -c[:, :, s])
    Cf = np.stack(planes, axis=0).astype(np.float16)  # [NT, O, I]

    # Per-core layouts.
    Cr = Cf.reshape(NT, O_SPLIT, O_LOC, 2, 128)       # [t, oh, o, ih, j]
    x16 = x.astype(np.float16)
    xr = x16.reshape(B_SPLIT, B_LOC, 2, 128)          # [bq, b, ih, j]
    bias16 = bias_o.astype(np.float16).reshape(O_SPLIT, O_LOC)

    in_maps = []
    for core in range(N_CORES):
        bq, oh = core % B_SPLIT, core // B_SPLIT
        xb_c = np.zeros((128, XB_COLS), np.float16)
        # xr[bq] axes (b, ih, j) -> (j, ih, b)
        xb_c[:, 0:FW] = xr[bq].transpose(2, 1, 0).reshape(128, FW)
        xb_c[:, FW] = bias16[oh]
        # C: [j, kt = 2t+ih, o]
        C_c = np.ascontiguousarray(
            Cr[:, oh].transpose(3, 0, 2, 1)).reshape(128, KT * 128)
        in_maps.append({"xb": np.ascontiguousarray(xb_c), "C": C_c})

    nc = _get_nc(scale, ucbias)
    return nc, in_maps


def kernel(x: np.ndarray, breakpoints: np.ndarray, values: np.ndarray,
           **_extra) -> np.ndarray:
    nc, in_maps = prepare(x, breakpoints, values)
    res = run_bass_kernel_spmd(nc, in_maps, list(range(N_CORES)))

    outf = np.empty((B, O), np.float32)
    for core in range(N_CORES):
        bq, oh = core % B_SPLIT, core // B_SPLIT
        outf[bq * B_LOC:(bq + 1) * B_LOC, oh * O_LOC:(oh + 1) * O_LOC] = \
            res.results[core]["out"].T.astype(np.float32)
    return outf


if __name__ == "__main__":
    rng = np.random.default_rng(0)
    x = rng.uniform(-1, 1, (B, I)).astype(np.float32)
    bp = np.tile(np.linspace(-1, 1, S + 1, dtype=np.float32), (O, I, 1))
    v = (rng.standard_normal((O, I, S + 1)) * 0.1).astype(np.float32)
    out = kernel(x, bp, v)
    print("kernel ran, out:", out.shape, out.dtype, float(out.std()))
